# revision 11
# baseline (speedup 1.0000x reference)
"""Trainium2 Bass kernel: single-head causal attention, data-parallel over 8 cores.

Problem shapes (hardcoded): x [512, 256, 384] f32, Wq/Wk/Wv [384, 64] f32.
Output: [512, 256, 64] f32 = softmax(causal(q @ k^T / 8)) @ v per batch.

Sharding: pure data parallel on batch: each of 8 cores gets 64 batches;
weights replicated. No collectives.

Per-core dataflow (all on-chip compute in fp16 with fp32 PSUM accumulation):
  - SWDGE cast-DMA loads x (fp32 HBM -> fp16 SBUF), natural [t, c] layout;
    x is fully buffered in SBUF so x-load DMAs carry no slot-reuse waits
    (the 64B instruction encodings only fit 1-2 sync waits; walrus rejects
    more, and multi-dim APs shrink the budget further - tiles are kept 2D).
  - TensorE transpose mode produces xT [c, t] (6x 128x128 per batch).
  - Projections: qT,kT = W.T @ xT (weights stationary, N=256);
    v = xT.T @ W in natural [t, h] layout (xT stationary, N=64).
  - Scores computed transposed: S'[s, t] = kT.T @ qT so softmax's
    reduction direction is handled by matmul (a ones-column appended to v
    yields the denominator); exp on ScalarE with the 1/sqrt(64) scale
    folded in; causal mask = 0/1 triangle multiply on GPSIMD.
  - out[t, h(+sum)] = P'.T @ [v | 1], then per-partition reciprocal scale.
"""

import os
from contextlib import ExitStack

import numpy as np

B, T, C, H = 512, 256, 384, 64
N_CORES = 8
B_LOCAL = B // N_CORES


def build_nc(b_local=B_LOCAL, group=4):
    import concourse.mybir as mybir
    import concourse.tile as tile
    from concourse import bacc

    F32 = mybir.dt.float32
    F16 = mybir.dt.float16
    AF = mybir.ActivationFunctionType
    ALU = mybir.AluOpType

    assert b_local % group == 0

    nc = bacc.Bacc()
    x = nc.declare_dram_parameter("x", [b_local, T, C], F32, isOutput=False)
    wq = nc.declare_dram_parameter("Wq", [C, H], F32, isOutput=False)
    wk = nc.declare_dram_parameter("Wk", [C, H], F32, isOutput=False)
    wv = nc.declare_dram_parameter("Wv", [C, H], F32, isOutput=False)
    out = nc.declare_dram_parameter("out", [b_local, T, H], F32, isOutput=True)

    NT = T // 128  # 2 token chunks
    NCC = C // 128  # 3 contraction chunks
    H1 = H + 1
    SCALE = 1.0 / np.sqrt(H)

    with tile.TileContext(nc) as tc, ExitStack() as ctx:
        const = ctx.enter_context(tc.tile_pool(name="const", bufs=1))
        xnat_p = ctx.enter_context(
            tc.tile_pool(name="xnat", bufs=b_local // group))
        xt_ps_p = ctx.enter_context(tc.tile_pool(name="xt_ps", bufs=2, space="PSUM"))
        xt_p = ctx.enter_context(tc.tile_pool(name="xt", bufs=3))
        qk_ps_p = ctx.enter_context(tc.tile_pool(name="qk_ps", bufs=2, space="PSUM"))
        qk_p = ctx.enter_context(tc.tile_pool(name="qk", bufs=2))
        v_ps_p = ctx.enter_context(tc.tile_pool(name="v_ps", bufs=2, space="PSUM"))
        v_p = ctx.enter_context(tc.tile_pool(name="v", bufs=2))
        s_ps_p = ctx.enter_context(tc.tile_pool(name="s_ps", bufs=1, space="PSUM"))
        p_p = ctx.enter_context(tc.tile_pool(name="p", bufs=2))
        o_ps_p = ctx.enter_context(tc.tile_pool(name="o_ps", bufs=1, space="PSUM"))
        o_p = ctx.enter_context(tc.tile_pool(name="o", bufs=2))
        r_p = ctx.enter_context(tc.tile_pool(name="r", bufs=2))

        # --- constants ---
        # fp16 weights, [128, chunk*H] with c = chunk*128 + partition.
        # Load fp32 via HWDGE (keeps the SWDGE sem lanes exclusive to x
        # loads), then cast on DVE.
        wq_sb = const.tile([128, NCC * H], F16, tag="wq")
        wk_sb = const.tile([128, NCC * H], F16, tag="wk")
        wv_sb = const.tile([128, NCC * H], F16, tag="wv")
        w_stage = const.tile([128, 3 * NCC * H], F32, tag="w_stage")
        for i, w in enumerate((wq, wk, wv)):
            nc.sync.dma_start(
                w_stage[:, i * NCC * H:(i + 1) * NCC * H],
                w.rearrange("(a p) h -> p a h", p=128))
        nc.vector.tensor_copy(wq_sb[:], w_stage[:, 0:NCC * H])
        nc.vector.tensor_copy(wk_sb[:], w_stage[:, NCC * H:2 * NCC * H])
        nc.vector.tensor_copy(wv_sb[:], w_stage[:, 2 * NCC * H:3 * NCC * H])

        ones = const.tile([128, 128], F16, tag="ones")
        nc.vector.memset(ones[:], 1.0)
        # tri[p, j] = 1 if j >= p else 0   (keep s <= t in S'[s, t] layout)
        tri = const.tile([128, 128], F16, tag="tri")
        nc.gpsimd.affine_select(
            tri[:], ones[:], pattern=[[1, 128]], compare_op=ALU.is_ge,
            fill=0.0, base=0, channel_multiplier=-1,
        )
        # identity for TensorE transpose
        ident = const.tile([128, 128], F16, tag="ident")
        nc.gpsimd.affine_select(
            ident[:], ones[:], pattern=[[1, 128]], compare_op=ALU.is_equal,
            fill=0.0, base=0, channel_multiplier=-1,
        )

        for g in range(b_local // group):
            # fp32 -> fp16 cast during DMA (SWDGE); x natural layout,
            # columns [(bb*NT + n)*C + c].
            xnat = xnat_p.tile([128, group * NT * C], F16, tag="xnat")
            nc.gpsimd.dma_start(
                xnat[:],
                x[g * group:(g + 1) * group].rearrange("b (n p) c -> p b n c", p=128),
            )
            for bb in range(group):
                b = g * group + bb
                # --- transpose x -> xT [c, t]; columns [cc*T + t] ---
                xt_ps = xt_ps_p.tile([128, NCC * T], F16, tag="xt_ps")
                for cc in range(NCC):
                    for n in range(NT):
                        nc.tensor.transpose(
                            xt_ps[:, cc * T + n * 128:cc * T + (n + 1) * 128],
                            xnat[:, (bb * NT + n) * C + cc * 128:
                                 (bb * NT + n) * C + (cc + 1) * 128],
                            ident[:],
                        )
                xt = xt_p.tile([128, NCC * T], F16, tag="xt")
                nc.vector.tensor_copy(xt[:], xt_ps[:])

                # --- projections ---
                # qT | kT side by side: [64, 0:256]=qT, [64, 256:512]=kT
                qk_ps = qk_ps_p.tile([H, 2 * T], F32, tag="qk_ps")
                v_ps = v_ps_p.tile([128, NT * H], F32, tag="v_ps")
                for cc in range(NCC):
                    st = dict(start=(cc == 0), stop=(cc == NCC - 1))
                    nc.tensor.matmul(
                        qk_ps[:, 0:T], wq_sb[:, cc * H:(cc + 1) * H],
                        xt[:, cc * T:(cc + 1) * T], **st)
                for cc in range(NCC):
                    st = dict(start=(cc == 0), stop=(cc == NCC - 1))
                    nc.tensor.matmul(
                        qk_ps[:, T:2 * T], wk_sb[:, cc * H:(cc + 1) * H],
                        xt[:, cc * T:(cc + 1) * T], **st)
                for n in range(NT):
                    for cc in range(NCC):
                        st = dict(start=(cc == 0), stop=(cc == NCC - 1))
                        nc.tensor.matmul(
                            v_ps[:, n * H:(n + 1) * H],
                            xt[:, cc * T + n * 128:cc * T + (n + 1) * 128],
                            wv_sb[:, cc * H:(cc + 1) * H], **st)

                qk = qk_p.tile([H, 2 * T], F16, tag="qk")
                nc.scalar.copy(qk[:], qk_ps[:])

                # v_ext = [v | 1]: ones column gives the softmax denominator
                vx = v_p.tile([128, NT * H1], F16, tag="vx")
                for n in range(NT):
                    nc.scalar.copy(
                        vx[:, n * H1:n * H1 + H], v_ps[:, n * H:(n + 1) * H])
                    nc.gpsimd.memset(vx[:, n * H1 + H:(n + 1) * H1], 1.0)

                # --- scores (transposed): S'[s, t] = kT.T @ qT ---
                # S0: s in [0,128), t in [0,256); S1: s in [128,256), t in [128,256)
                s_ps = s_ps_p.tile([128, T + 128], F32, tag="s_ps")
                nc.tensor.matmul(s_ps[:, 0:T], qk[:, T:T + 128], qk[:, 0:T])
                nc.tensor.matmul(s_ps[:, T:T + 128], qk[:, T + 128:2 * T],
                                 qk[:, 128:T])

                # --- exp (scale folded in); causal mask on diagonal blocks ---
                p0 = p_p.tile([128, T], F16, tag="p0")
                p1 = p_p.tile([128, 128], F16, tag="p1")
                nc.scalar.activation(p0[:], s_ps[:, 0:T], AF.Exp, scale=SCALE)
                nc.scalar.activation(p1[:], s_ps[:, T:T + 128], AF.Exp, scale=SCALE)
                nc.gpsimd.tensor_mul(p0[:, 0:128], p0[:, 0:128], tri[:])
                nc.gpsimd.tensor_mul(p1[:], p1[:], tri[:])

                # --- out[t, h(+denominator)] = P'.T @ v_ext ---
                o_ps = o_ps_p.tile([128, NT * H1], F32, tag="o_ps")
                nc.tensor.matmul(o_ps[:, 0:H1], p0[:, 0:128], vx[:, 0:H1])
                nc.tensor.matmul(o_ps[:, H1:2 * H1], p0[:, 128:T], vx[:, 0:H1],
                                 start=True, stop=False)
                nc.tensor.matmul(o_ps[:, H1:2 * H1], p1[:], vx[:, H1:2 * H1],
                                 start=False, stop=True)

                # --- normalize: out / denominator, write fp32 ---
                rec = r_p.tile([128, NT], F32, tag="rec")
                nc.vector.reciprocal(rec[:], o_ps[:, H::H1])
                ob = o_p.tile([128, NT * H], F32, tag="ob")
                for n in range(NT):
                    nc.vector.tensor_scalar_mul(
                        ob[:, n * H:(n + 1) * H],
                        o_ps[:, n * H1:n * H1 + H],
                        rec[:, n:n + 1])
                    nc.sync.dma_start(
                        out[b, n * 128:(n + 1) * 128, :],
                        ob[:, n * H:(n + 1) * H])

    nc.compile()
    return nc


_CACHED = {}


def _get_nc():
    if "nc" not in _CACHED:
        _CACHED["nc"] = build_nc()
    return _CACHED["nc"]


def kernel(x, Wq, Wk, Wv):
    from concourse.bass_utils import run_bass_kernel_spmd

    nc = _get_nc()
    x = np.ascontiguousarray(x, dtype=np.float32)
    in_maps = [
        {
            "x": x[i * B_LOCAL:(i + 1) * B_LOCAL],
            "Wq": np.asarray(Wq, dtype=np.float32),
            "Wk": np.asarray(Wk, dtype=np.float32),
            "Wv": np.asarray(Wv, dtype=np.float32),
        }
        for i in range(N_CORES)
    ]
    res = run_bass_kernel_spmd(
        nc, in_maps, core_ids=list(range(N_CORES)),
        trace=bool(int(os.environ.get("KERNEL_TRACE", "0"))),
    )
    out = np.concatenate([r["out"] for r in res.results], axis=0)
    _CACHED["last_result"] = res
    return out


# revision 14
# speedup vs baseline: 24555.3037x; 24555.3037x over previous
"""Trainium2 Bass kernel: single-head causal attention, data-parallel over 8 cores.

Problem shapes (hardcoded): x [512, 256, 384] f32, Wq/Wk/Wv [384, 64] f32.
Output: [512, 256, 64] f32 = softmax(causal(q @ k^T / 8)) @ v per batch.

Sharding: pure data parallel on batch: each of 8 cores gets 64 batches;
weights replicated. No collectives.

Per-core dataflow (all on-chip compute in fp16 with fp32 PSUM accumulation):
  - SWDGE cast-DMA loads x (fp32 HBM -> fp16 SBUF), natural [t, c] layout;
    x is fully buffered in SBUF so x-load DMAs carry no slot-reuse waits
    (the 64B instruction encodings only fit 1-2 sync waits; walrus rejects
    more, and multi-dim APs shrink the budget further - tiles are kept 2D).
  - TensorE transpose mode produces xT [c, t] (6x 128x128 per batch).
  - Projections: qT,kT = W.T @ xT (weights stationary, N=256);
    v = xT.T @ W in natural [t, h] layout (xT stationary, N=64).
  - Scores computed transposed: S'[s, t] = kT.T @ qT so softmax's
    reduction direction is handled by matmul (a ones-column appended to v
    yields the denominator); exp on ScalarE with the 1/sqrt(64) scale
    folded in; causal mask = 0/1 triangle multiply on GPSIMD.
  - out[t, h(+sum)] = P'.T @ [v | 1], then per-partition reciprocal scale.
"""

import os
from contextlib import ExitStack

import numpy as np

B, T, C, H = 512, 256, 384, 64
N_CORES = 8
B_LOCAL = B // N_CORES


def build_nc(b_local=B_LOCAL, group=4):
    import concourse.mybir as mybir
    import concourse.tile as tile
    from concourse import bacc

    F32 = mybir.dt.float32
    F16 = mybir.dt.float16
    AF = mybir.ActivationFunctionType
    ALU = mybir.AluOpType

    assert b_local % group == 0

    nc = bacc.Bacc()
    x = nc.declare_dram_parameter("x", [b_local, T, C], F32, isOutput=False)
    wq = nc.declare_dram_parameter("Wq", [C, H], F32, isOutput=False)
    wk = nc.declare_dram_parameter("Wk", [C, H], F32, isOutput=False)
    wv = nc.declare_dram_parameter("Wv", [C, H], F32, isOutput=False)
    out = nc.declare_dram_parameter("out", [b_local, T, H], F32, isOutput=True)

    NT = T // 128  # 2 token chunks
    NCC = C // 128  # 3 contraction chunks
    H1 = H + 1
    SCALE = 1.0 / np.sqrt(H)

    with tile.TileContext(nc) as tc, ExitStack() as ctx:
        const = ctx.enter_context(tc.tile_pool(name="const", bufs=1))
        xnat_p = ctx.enter_context(
            tc.tile_pool(name="xnat", bufs=b_local // group))
        xt_ps_p = ctx.enter_context(tc.tile_pool(name="xt_ps", bufs=2, space="PSUM"))
        xt_p = ctx.enter_context(tc.tile_pool(name="xt", bufs=3))
        qk_ps_p = ctx.enter_context(tc.tile_pool(name="qk_ps", bufs=2, space="PSUM"))
        qk_p = ctx.enter_context(tc.tile_pool(name="qk", bufs=2))
        v_ps_p = ctx.enter_context(tc.tile_pool(name="v_ps", bufs=2, space="PSUM"))
        v_p = ctx.enter_context(tc.tile_pool(name="v", bufs=2))
        s_ps_p = ctx.enter_context(tc.tile_pool(name="s_ps", bufs=1, space="PSUM"))
        p_p = ctx.enter_context(tc.tile_pool(name="p", bufs=2))
        o_ps_p = ctx.enter_context(tc.tile_pool(name="o_ps", bufs=1, space="PSUM"))
        o_p = ctx.enter_context(tc.tile_pool(name="o", bufs=2))
        r_p = ctx.enter_context(tc.tile_pool(name="r", bufs=2))

        # --- constants ---
        # fp16 weights, [128, chunk*H] with c = chunk*128 + partition.
        # Load fp32 via HWDGE (keeps the SWDGE sem lanes exclusive to x
        # loads), then cast on DVE.
        wq_sb = const.tile([128, NCC * H], F16, tag="wq")
        wk_sb = const.tile([128, NCC * H], F16, tag="wk")
        wv_sb = const.tile([128, NCC * H], F16, tag="wv")
        w_stage = const.tile([128, 3 * NCC * H], F32, tag="w_stage")
        for i, w in enumerate((wq, wk, wv)):
            nc.sync.dma_start(
                w_stage[:, i * NCC * H:(i + 1) * NCC * H],
                w.rearrange("(a p) h -> p a h", p=128))
        nc.vector.tensor_copy(wq_sb[:], w_stage[:, 0:NCC * H])
        nc.vector.tensor_copy(wk_sb[:], w_stage[:, NCC * H:2 * NCC * H])
        nc.vector.tensor_copy(wv_sb[:], w_stage[:, 2 * NCC * H:3 * NCC * H])

        ones = const.tile([128, 128], F16, tag="ones")
        nc.vector.memset(ones[:], 1.0)
        # tri[p, j] = 1 if j >= p else 0   (keep s <= t in S'[s, t] layout)
        tri = const.tile([128, 128], F16, tag="tri")
        nc.gpsimd.affine_select(
            tri[:], ones[:], pattern=[[1, 128]], compare_op=ALU.is_ge,
            fill=0.0, base=0, channel_multiplier=-1,
        )
        # identity for TensorE transpose
        ident = const.tile([128, 128], F16, tag="ident")
        nc.gpsimd.affine_select(
            ident[:], ones[:], pattern=[[1, 128]], compare_op=ALU.is_equal,
            fill=0.0, base=0, channel_multiplier=-1,
        )

        for g in range(b_local // group):
            # fp32 -> fp16 cast during DMA (SWDGE); x natural layout,
            # columns [(bb*NT + n)*C + c].
            xnat = xnat_p.tile([128, group * NT * C], F16, tag="xnat")
            nc.gpsimd.dma_start(
                xnat[:],
                x[g * group:(g + 1) * group].rearrange("b (n p) c -> p b n c", p=128),
            )
            for bb in range(group):
                b = g * group + bb
                # --- transpose x -> xT [c, t]; columns [cc*T + t] ---
                xt_ps = xt_ps_p.tile([128, NCC * T], F16, tag="xt_ps")
                for cc in range(NCC):
                    for n in range(NT):
                        nc.tensor.transpose(
                            xt_ps[:, cc * T + n * 128:cc * T + (n + 1) * 128],
                            xnat[:, (bb * NT + n) * C + cc * 128:
                                 (bb * NT + n) * C + (cc + 1) * 128],
                            ident[:],
                        )
                xt = xt_p.tile([128, NCC * T], F16, tag="xt")
                nc.vector.tensor_copy(xt[:], xt_ps[:])

                # --- projections ---
                # qT | kT side by side: [64, 0:256]=qT, [64, 256:512]=kT
                qk_ps = qk_ps_p.tile([H, 2 * T], F32, tag="qk_ps")
                v_ps = v_ps_p.tile([128, NT * H], F32, tag="v_ps")
                for cc in range(NCC):
                    st = dict(start=(cc == 0), stop=(cc == NCC - 1))
                    nc.tensor.matmul(
                        qk_ps[:, 0:T], wq_sb[:, cc * H:(cc + 1) * H],
                        xt[:, cc * T:(cc + 1) * T], **st)
                for cc in range(NCC):
                    st = dict(start=(cc == 0), stop=(cc == NCC - 1))
                    nc.tensor.matmul(
                        qk_ps[:, T:2 * T], wk_sb[:, cc * H:(cc + 1) * H],
                        xt[:, cc * T:(cc + 1) * T], **st)
                for n in range(NT):
                    for cc in range(NCC):
                        st = dict(start=(cc == 0), stop=(cc == NCC - 1))
                        nc.tensor.matmul(
                            v_ps[:, n * H:(n + 1) * H],
                            xt[:, cc * T + n * 128:cc * T + (n + 1) * 128],
                            wv_sb[:, cc * H:(cc + 1) * H], **st)

                qk = qk_p.tile([H, 2 * T], F16, tag="qk")
                nc.scalar.copy(qk[:], qk_ps[:])

                # v_ext = [v | 1]: ones column gives the softmax denominator
                vx = v_p.tile([128, NT * H1], F16, tag="vx")
                for n in range(NT):
                    nc.scalar.copy(
                        vx[:, n * H1:n * H1 + H], v_ps[:, n * H:(n + 1) * H])
                    nc.gpsimd.memset(vx[:, n * H1 + H:(n + 1) * H1], 1.0)

                # --- scores (transposed): S'[s, t] = kT.T @ qT ---
                # S0: s in [0,128), t in [0,256); S1: s in [128,256), t in [128,256)
                s_ps = s_ps_p.tile([128, T + 128], F32, tag="s_ps")
                nc.tensor.matmul(s_ps[:, 0:T], qk[:, T:T + 128], qk[:, 0:T])
                nc.tensor.matmul(s_ps[:, T:T + 128], qk[:, T + 128:2 * T],
                                 qk[:, 128:T])

                # --- exp (scale folded in); causal mask on diagonal blocks ---
                p0 = p_p.tile([128, T], F16, tag="p0")
                p1 = p_p.tile([128, 128], F16, tag="p1")
                nc.scalar.activation(p0[:], s_ps[:, 0:T], AF.Exp, scale=SCALE)
                nc.scalar.activation(p1[:], s_ps[:, T:T + 128], AF.Exp, scale=SCALE)
                nc.gpsimd.tensor_mul(p0[:, 0:128], p0[:, 0:128], tri[:])
                nc.gpsimd.tensor_mul(p1[:], p1[:], tri[:])

                # --- out[t, h(+denominator)] = P'.T @ v_ext ---
                o_ps = o_ps_p.tile([128, NT * H1], F32, tag="o_ps")
                nc.tensor.matmul(o_ps[:, 0:H1], p0[:, 0:128], vx[:, 0:H1])
                nc.tensor.matmul(o_ps[:, H1:2 * H1], p0[:, 128:T], vx[:, 0:H1],
                                 start=True, stop=False)
                nc.tensor.matmul(o_ps[:, H1:2 * H1], p1[:], vx[:, H1:2 * H1],
                                 start=False, stop=True)

                # --- normalize: out / denominator, write fp32 ---
                rec = r_p.tile([128, NT], F32, tag="rec")
                nc.vector.reciprocal(rec[:], o_ps[:, H::H1])
                ob = o_p.tile([128, NT * H], F32, tag="ob")
                for n in range(NT):
                    nc.vector.tensor_scalar_mul(
                        ob[:, n * H:(n + 1) * H],
                        o_ps[:, n * H1:n * H1 + H],
                        rec[:, n:n + 1])
                    nc.sync.dma_start(
                        out[b, n * 128:(n + 1) * 128, :],
                        ob[:, n * H:(n + 1) * H])

    nc.compile()
    return nc


_CACHED = {}


def _make_runner(nc):
    """Build a cached shard_map'd jit for an SPMD Bass program.

    Mirrors concourse.bass2jax.run_bass_via_pjrt, but without output-buffer
    donation so the compiled executable can be re-invoked with
    device-resident arrays (no host transfers on warm calls).
    """
    import jax
    from jax.experimental.shard_map import shard_map
    from jax.sharding import Mesh, NamedSharding, PartitionSpec

    import concourse.mybir as mybir
    from concourse.bass2jax import (
        _bass_exec_p, install_neuronx_cc_hook, partition_id_tensor)

    install_neuronx_cc_hook()

    partition_name = (
        nc.partition_id_tensor.name if nc.partition_id_tensor else None)
    in_names, out_names, out_avals, zero_outs = [], [], [], []
    for alloc in nc.m.functions[0].allocations:
        if not isinstance(alloc, mybir.MemoryLocationSet):
            continue
        name = alloc.memorylocations[0].name
        if alloc.kind == "ExternalInput":
            if name != partition_name:
                in_names.append(name)
        elif alloc.kind == "ExternalOutput":
            out_names.append(name)
            shape = tuple(alloc.tensor_shape)
            dtype = mybir.dt.np(alloc.dtype)
            out_avals.append(jax.core.ShapedArray(shape, dtype))
            zero_outs.append(np.zeros(shape, dtype))
    n_params = len(in_names)
    all_in = in_names + out_names
    if partition_name is not None:
        all_in = all_in + [partition_name]

    def _body(*args):
        operands = list(args)
        if partition_name is not None:
            operands.append(partition_id_tensor())
        outs = _bass_exec_p.bind(
            *operands,
            out_avals=tuple(out_avals),
            in_names=tuple(all_in),
            out_names=tuple(out_names),
            lowering_input_output_aliases=(),
            sim_require_finite=False,
            sim_require_nnan=False,
            nc=nc,
        )
        return tuple(outs)

    devices = jax.devices()[:N_CORES]
    mesh = Mesh(np.asarray(devices), ("core",))
    spec = PartitionSpec("core")
    n_args = n_params + len(out_names)
    sharded = jax.jit(
        shard_map(
            _body, mesh=mesh, in_specs=(spec,) * n_args,
            out_specs=(spec,) * len(out_names), check_rep=False,
        ),
        keep_unused=True,
    )
    sharding = NamedSharding(mesh, spec)
    return sharded, in_names, zero_outs, sharding


def _get_runner():
    if "runner" not in _CACHED:
        _CACHED["runner"] = _make_runner(build_nc())
    return _CACHED["runner"]


def _device_inputs(x, Wq, Wk, Wv):
    """Concat per-core inputs on axis 0 (shard_map layout) and device_put."""
    import jax

    sharded, in_names, zero_outs, sharding = _get_runner()
    x = np.ascontiguousarray(x, dtype=np.float32)
    assert x.shape == (B, T, C)
    host = {
        "x": x,
        "Wq": np.concatenate([np.asarray(Wq, np.float32)] * N_CORES, axis=0),
        "Wk": np.concatenate([np.asarray(Wk, np.float32)] * N_CORES, axis=0),
        "Wv": np.concatenate([np.asarray(Wv, np.float32)] * N_CORES, axis=0),
    }
    args = [host[n] for n in in_names]
    args += [
        np.zeros((N_CORES * z.shape[0], *z.shape[1:]), z.dtype) for z in zero_outs
    ]
    return [jax.device_put(a, sharding) for a in args]


def kernel(x, Wq, Wk, Wv):
    sharded, _, _, _ = _get_runner()
    args = _device_inputs(x, Wq, Wk, Wv)
    (out,) = sharded(*args)
    return np.asarray(out)
